# revision 19
# baseline (speedup 1.0000x reference)
"""Trainium2 Bass kernel for CapsuleLayer dynamic routing (v2).

Problem: x [64, 2048, 16], W [1, 2048, 32, 32, 16] ->
  u_hat = einsum('bik,ijdk->bijd', x, W[0])           [B, N_in, N_out, D_out]
  3 rounds of routing (softmax over j, weighted sum over i, squash),
  returns v [64, 32, 32].

Sharding: N_in (2048) split over 8 cores, 256 input capsules each; per-round
partial weighted sums are AllReduced; squash/softmax replicated.

Per-core design (derived from a perfetto-trace cost model of the v1 kernel,
which was DVE-bound at 81% occupancy):
  * Batch processed in 2 chunks of 32 (u_hat chunk = 16.8MB bf16 in SBUF).
  * U layout [p=(q4, b32); (blk32, g2 2, d32, j32)] with j INNERMOST:
    both big per-round multiplies (U*v-broadcast and U*c-broadcast) hit the
    DVE 2x_1P mode (broadcast strides live on non-innermost dims).
  * All big reductions are TT-add TREES at 2x instead of native
    tensor_reduce (which is capped at 1x).
  * Phase 1 computes u_hat with 8-way 32x32 PE-array tile packing
    (row bands (g2,gp) x col bands (gp,h)), sharing one W stream per blk
    with the K=128 full-array s0 matmuls (t=0 shortcut, keeps HAM warm).
  * PSUM->SBUF drains split between DVE (CAST 2x) and ACT.
  * Final (sec, q) reduction of weighted-sum partials via a ones-matmul on
    the PE with psum accumulation.
  * squash uses only exp/ln (one ACT table set): rsqrt(x) = exp(-0.5*ln(x)).
"""
import sys

sys.path.insert(0, '/opt/trn_rl_repo')

import numpy as np

import concourse.bass as bass
import concourse.mybir as mybir
from concourse import bass_utils, tile

# ---------------------------------------------------------------- constants
N_CORES = 8
B = 64
N_IN = 2048
D_IN = 16
N_OUT = 32
D_OUT = 32
ROUTINGS = 3
EPS = 1e-9

I_LOC = N_IN // N_CORES          # 256 local capsules
NBLK = I_LOC // 8                # 32 blocks of 8 capsules
BC = 32                          # batch chunk
NCHUNK = B // BC                 # 2
JD = N_OUT * D_OUT               # 1024 (d,j) values per capsule
NSEC = 8                         # routing sections per chunk-round
SEC_BLKS = NBLK // NSEC          # 4 blocks per section
SEC_BH = SEC_BLKS * 2            # 8 (blk, g2) groups per section
SEC_COLS = SEC_BH * JD           # 8192 U-columns per section

f32 = mybir.dt.float32
bf16 = mybir.dt.bfloat16

_MAX_WAITS = 1
_carrier = [0]


def _patch_tile():
    """Work around this walrus build rejecting >1 sync wait per instruction."""
    import concourse.mybir as _mybir
    from concourse import tile as _tile
    from concourse.tile import TileContext as _TC

    def _drain_and_barrier(self, tick_clock, wait_clock):
        ScopedClock = _tile.ScopedClock
        probe = self.nc.sync.nop(nofuse=True)
        wait_clock.add_sem_waits(
            probe.ins, ScopedClock({None: tick_clock.global_clock})
        )
        si = probe.ins.sync_info
        waits = list(si.on_wait)
        probe.ins.sync_info = _mybir.SyncInfo(
            on_wait=waits[:1], on_update=list(si.on_update)
        )
        for w in waits[1:]:
            carrier = self.nc.sync.nop(nofuse=True)
            carrier.ins.sync_info = _mybir.SyncInfo(on_wait=[w], on_update=[])
        self.nc.sync.drain()
        self.nc.all_engine_barrier()
        assert self.sems is not None
        popped = self.nc._tile_sem_poison_stack.pop()
        assert popped is self._sem_poison
        self.nc.clear_and_free_semaphores(list(self.sems.allocated().values()))
        self.nc.all_engine_barrier()

    _TC._drain_and_barrier = _drain_and_barrier

    try:
        from concourse import tile_utils
        tile_utils.max_sbuf_usage = 208 * 1024
    except Exception:
        pass


def _fix_sync_waits(nc, max_waits=_MAX_WAITS):
    n_fixed = 0
    for func in nc.m.functions:
        for bb in func.blocks:
            insts = list(bb.instructions)
            new_list = []
            changed = False
            for inst in insts:
                si = getattr(inst, "sync_info", None)
                waits = list(si.on_wait) if si is not None else []
                if len(waits) > max_waits:
                    keep = waits[: max_waits - 1] if max_waits > 1 else []
                    hoist = waits[len(keep):-1]
                    tail = [waits[-1]]
                    for w in hoist:
                        _carrier[0] += 1
                        nop = mybir.InstNoOp(
                            name=f"syncfix-{_carrier[0]}", engine=inst.engine
                        )
                        nop.sync_info = mybir.SyncInfo(on_wait=[w], on_update=[])
                        new_list.append(nop)
                    inst.sync_info = mybir.SyncInfo(
                        on_wait=keep + tail, on_update=list(si.on_update)
                    )
                    changed = True
                    n_fixed += 1
                new_list.append(inst)
            if changed:
                bb.instructions = new_list
    return n_fixed


# ---------------------------------------------------------------- program
def _build_program():
    _patch_tile()
    nc = bass.Bass(trn_type="TRN2", num_devices=N_CORES)

    wt_in = nc.dram_tensor("wt", [128, NBLK * JD], bf16, kind="ExternalInput")
    xz_in = nc.dram_tensor("xz", [128, NBLK * 2 * B], bf16, kind="ExternalInput")
    xd_in = nc.dram_tensor("xd", [128, NBLK * B], bf16, kind="ExternalInput")
    ones_in = nc.dram_tensor("ones32", [128, BC], bf16, kind="ExternalInput")
    v_out = nc.dram_tensor("v", [B, JD], f32, kind="ExternalOutput")

    AluOp = mybir.AluOpType
    Act = mybir.ActivationFunctionType
    Axis = mybir.AxisListType
    rg = [list(range(N_CORES))]

    from contextlib import ExitStack
    with tile.TileContext(nc, num_cores=N_CORES) as tc, ExitStack() as es:
        cpool = es.enter_context(tc.tile_pool(name="const", bufs=1))
        wpool = es.enter_context(tc.tile_pool(name="wstream", bufs=5))
        upool = es.enter_context(tc.tile_pool(name="ubuf", bufs=1))
        scpool = es.enter_context(tc.tile_pool(name="scratch", bufs=1))
        smpool = es.enter_context(tc.tile_pool(name="small", bufs=1))
        ps1pool = es.enter_context(tc.tile_pool(name="psph1", bufs=2, space="PSUM"))
        ps0pool = es.enter_context(tc.tile_pool(name="psums0", bufs=1, space="PSUM"))
        psrpool = es.enter_context(tc.tile_pool(name="psred", bufs=1, space="PSUM"))
        dpool = es.enter_context(tc.tile_pool(name="dram", bufs=1, space="DRAM"))

        # ---- constants / inputs resident in SBUF
        epsc = cpool.tile([128, 1], f32, tag="epsc")
        nc.vector.memset(epsc[:], EPS)
        xz = cpool.tile([128, NBLK * 2 * B], bf16)
        xd = cpool.tile([128, NBLK * B], bf16)
        ones32 = cpool.tile([128, BC], bf16)
        nc.sync.dma_start(xd[:], xd_in[:])
        nc.sync.dma_start(ones32[:], ones_in[:])

        # ---- big buffers
        U = upool.tile([128, NBLK * 2 * JD], bf16, tag="U")       # 128KB/p
        prodb = scpool.tile([128, SEC_COLS], bf16, tag="prod")    # 16KB/p
        treeb = scpool.tile([128, SEC_COLS // 2], bf16, tag="tree")  # 8KB/p
        algA = smpool.tile([128, NBLK * 2 * N_OUT], f32, tag="algA")  # 8KB/p
        cbuf = smpool.tile([128, NBLK * 2 * N_OUT], bf16, tag="c")
        Zt = smpool.tile([128, NBLK * 2], f32, tag="Z")
        Zr = smpool.tile([128, NBLK * 2], f32, tag="Zr")
        s_rep = smpool.tile([128, JD], f32, tag="srep")
        s2 = smpool.tile([128, N_OUT], f32, tag="s2")
        lns = smpool.tile([128, N_OUT], f32, tag="lns")
        rsq = smpool.tile([128, N_OUT], f32, tag="rsq")
        den = smpool.tile([128, N_OUT], f32, tag="den")
        rinv = smpool.tile([128, N_OUT], f32, tag="rinv")
        fsc = smpool.tile([128, N_OUT], f32, tag="fsc")
        ibuf = smpool.tile([128, N_OUT], mybir.dt.int32, tag="ibuf")
        v_bf = smpool.tile([128, JD], bf16, tag="vbf")
        s_sb = smpool.tile([BC, JD], f32, tag="ssb")
        s0_sb = smpool.tile([B, JD], f32, tag="s0sb")
        v_fin = smpool.tile([BC, JD], f32, tag="vfin")

        # aliased scratch views (prodb/treeb free at those times)
        ebuf = prodb[:, :4096].bitcast(f32)          # [128, 2048] exp(logits)
        p2 = prodb[:, 4096:6144].bitcast(f32)        # [128, 1024] squash s^2
        tsec = treeb[:, 2048:2560].bitcast(f32)      # [128, 256] t=2 A-slice

        ps_s0 = ps0pool.tile([B, JD], f32, tag="pss0")

        # AR dram staging
        ar0_in = dpool.tile([B, JD], f32, tag="ar0i")
        ar0_out = dpool.tile([B, JD], f32, tag="ar0o")
        ar_bufs = {}
        for q in range(NCHUNK):
            for t in (1, 2):
                ar_bufs[(q, t)] = (
                    dpool.tile([BC, JD], f32, name=f"ari{q}{t}"),
                    dpool.tile([BC, JD], f32, name=f"aro{q}{t}"),
                )

        # ------------------------------------------------------ phase 1
        def warmup():
            """~6us of back-to-back full-array matmuls to flip the PE HAM
            clock gate to 8/8 before a phase-1 burst (output never read)."""
            ps = ps1pool.tile([128, JD], f32, tag="ps1", name="warm")
            for i in range(14):
                nc.tensor.matmul(
                    ps[0:64, 0:512], xd[:, 0:B], xz[:, 0:512],
                    start=True, stop=True,
                )

        def s0_burst():
            """t=0 shortcut: s0 = (1/32) sum_i u_hat, full batch, K=128
            full-array matmuls, own W stream; runs before phase 1 so the
            AllReduce of s0 completes early (also warms the PE HAM)."""
            for blk in range(NBLK):
                w = wpool.tile([128, JD], bf16, tag="w", name="w0")
                nc.sync.dma_start(w[:], wt_in[:, blk * JD:(blk + 1) * JD])
                for half in range(2):
                    nc.tensor.matmul(
                        ps_s0[:, half * 512:(half + 1) * 512],
                        xd[:, blk * B:(blk + 1) * B],
                        w[:, half * 512:(half + 1) * 512],
                        start=(blk == 0), stop=(blk == NBLK - 1),
                    )

        def phase1(q, agree_lag=None, pre_agree=None, dve_drain_mod=0):
            """u_hat for batch chunk q into U.  If agree_lag is not None,
            the round-1 agreement sections are emitted inline, lagged by
            `agree_lag` phase-1 sections, to fill the DVE idle window while
            the PE streams the tiled matmuls.  pre_agree() is emitted right
            before the first inlined agreement section.  A drain goes to the
            DVE when (idx % 4) < dve_drain_mod, else to ACT."""
            emitted_pre = [False]
            for blk in range(NBLK):
                w = wpool.tile([128, JD], bf16, tag="w")
                nc.sync.dma_start(w[:], wt_in[:, blk * JD:(blk + 1) * JD])
                for g2 in range(2):
                    ps = ps1pool.tile([128, JD], f32, tag="ps1")
                    for gp in range(2):
                        r = g2 * 2 + gp
                        for h in range(2):
                            m = gp * 2 + h
                            lhs = xz[32 * r:32 * r + 32,
                                     blk * 2 * B + h * B + q * BC:
                                     blk * 2 * B + h * B + q * BC + BC]
                            for half in range(2):
                                nc.tensor.matmul(
                                    ps[32 * m:32 * m + 32,
                                       half * 512:(half + 1) * 512],
                                    lhs,
                                    w[32 * r:32 * r + 32,
                                      half * 512:(half + 1) * 512],
                                    start=True, stop=True,
                                    tile_position=(32 * r, 32 * m),
                                )
                    dst = U[:, blk * 2 * JD + g2 * JD:
                            blk * 2 * JD + (g2 + 1) * JD]
                    if (blk * 2 + g2) % 4 < dve_drain_mod:
                        nc.vector.tensor_copy(dst, ps[:])
                    else:
                        nc.scalar.copy(dst, ps[:])
                if agree_lag is not None and (blk + 1) % SEC_BLKS == 0:
                    p1sec = (blk + 1) // SEC_BLKS - 1
                    asec = p1sec - agree_lag
                    if asec >= 0:
                        if not emitted_pre[0]:
                            emitted_pre[0] = True
                            if pre_agree is not None:
                                pre_agree()
                        agreement_section(1, asec)
            if agree_lag is not None:
                for asec in range(NSEC - agree_lag, NSEC):
                    agreement_section(1, asec)

        # ------------------------------------------------------ routing ops
        def agreement_section(t, sec):
                Us = U[:, sec * SEC_COLS:(sec + 1) * SEC_COLS].rearrange(
                    "p (bh d j) -> p bh d j", bh=SEC_BH, d=D_OUT, j=N_OUT)
                P = prodb[:].rearrange(
                    "p (bh d j) -> p bh d j", bh=SEC_BH, d=D_OUT, j=N_OUT)
                v4 = (v_bf[:]
                      .rearrange("p (d j) -> p d j", d=D_OUT, j=N_OUT)
                      .unsqueeze(1)
                      .to_broadcast((128, SEC_BH, D_OUT, N_OUT)))
                nc.vector.tensor_tensor(P, Us, v4, AluOp.mult)   # 2x
                # d-tree: 32 -> 16 -> 8 -> 4 -> 2 -> 1
                T1 = treeb[:, :4096].rearrange(
                    "p (bh d j) -> p bh d j", bh=SEC_BH, d=16, j=N_OUT)
                nc.vector.tensor_tensor(
                    T1, P[:, :, 0:16, :], P[:, :, 16:32, :], AluOp.add)
                T2 = prodb[:, :2048].rearrange(
                    "p (bh d j) -> p bh d j", bh=SEC_BH, d=8, j=N_OUT)
                nc.vector.tensor_tensor(
                    T2, T1[:, :, 0:8, :], T1[:, :, 8:16, :], AluOp.add)
                T3 = treeb[:, :1024].rearrange(
                    "p (bh d j) -> p bh d j", bh=SEC_BH, d=4, j=N_OUT)
                nc.vector.tensor_tensor(
                    T3, T2[:, :, 0:4, :], T2[:, :, 4:8, :], AluOp.add)
                T4 = prodb[:, 2048:2560].rearrange(
                    "p (bh d j) -> p bh d j", bh=SEC_BH, d=2, j=N_OUT)
                nc.vector.tensor_tensor(
                    T4, T3[:, :, 0:2, :], T3[:, :, 2:4, :], AluOp.add)
                a_cols = SEC_BH * N_OUT            # 256 logit cols / section
                if t == 1:
                    T5 = algA[:, sec * a_cols:(sec + 1) * a_cols].rearrange(
                        "p (bh d j) -> p bh d j", bh=SEC_BH, d=1, j=N_OUT)
                    nc.vector.tensor_tensor(
                        T5, T4[:, :, 0:1, :], T4[:, :, 1:2, :], AluOp.add)
                else:
                    T5 = tsec.rearrange(
                        "p (bh d j) -> p bh d j", bh=SEC_BH, d=1, j=N_OUT)
                    nc.vector.tensor_tensor(
                        T5, T4[:, :, 0:1, :], T4[:, :, 1:2, :], AluOp.add)
                    asl = algA[:, sec * a_cols:(sec + 1) * a_cols]
                    nc.vector.tensor_add(asl, asl, tsec)

        def softmax():
            nc.scalar.activation(ebuf, algA[:], Act.Exp)
            e3 = ebuf.rearrange("p (bh j) -> p bh j", bh=NBLK * 2, j=N_OUT)
            nc.vector.reduce_sum(Zt[:], e3, axis=Axis.X)
            nc.vector.reciprocal(Zr[:], Zt[:])
            c3 = cbuf[:].rearrange("p (bh j) -> p bh j", bh=NBLK * 2, j=N_OUT)
            zr3 = Zr[:].unsqueeze(2).to_broadcast((128, NBLK * 2, N_OUT))
            nc.vector.tensor_tensor(c3, e3, zr3, AluOp.mult)

        def weighted_sum(q, t):
            """s_partial = sum_i c * U.  The DVE only computes the c*U
            product (2x); ALL the summation (64 bh column-groups AND the 4
            partition groups) happens on the PE as one 128-matmul psum
            accumulation chain through the ones matrix."""
            ps_red = psrpool.tile([BC, JD], f32, tag="psred")
            NSW = 16                       # 2-blk product sections
            SW_BH = NBLK * 2 // NSW        # 4 (blk,g2) groups per section
            SW_COLS = SW_BH * JD           # 4096
            slots = [prodb[:, 0:SW_COLS], prodb[:, SW_COLS:2 * SW_COLS],
                     treeb[:, 0:SW_COLS]]
            for sec in range(NSW):
                slot = slots[sec % 3]
                P = slot.rearrange(
                    "p (bh d j) -> p bh d j", bh=SW_BH, d=D_OUT, j=N_OUT)
                Us = U[:, sec * SW_COLS:(sec + 1) * SW_COLS].rearrange(
                    "p (bh d j) -> p bh d j", bh=SW_BH, d=D_OUT, j=N_OUT)
                a_cols = SW_BH * N_OUT
                c4 = (cbuf[:, sec * a_cols:(sec + 1) * a_cols]
                      .rearrange("p (bh j) -> p bh j", bh=SW_BH, j=N_OUT)
                      .unsqueeze(2)
                      .to_broadcast((128, SW_BH, D_OUT, N_OUT)))
                nc.vector.tensor_tensor(P, Us, c4, AluOp.mult)   # 2x
                for bh in range(SW_BH):
                    for half in range(2):
                        nc.tensor.matmul(
                            ps_red[:, half * 512:(half + 1) * 512],
                            ones32[:],
                            slot[:, bh * JD + half * 512:
                                 bh * JD + half * 512 + 512],
                            start=(sec == 0 and bh == 0),
                            stop=(sec == NSW - 1 and bh == SW_BH - 1),
                        )
            nc.scalar.copy(s_sb[:], ps_red[:])
            ar_in, ar_out = ar_bufs[(q, t)]
            nc.gpsimd.dma_start(ar_in[:], s_sb[:])
            nc.gpsimd.collective_compute(
                "AllReduce", AluOp.add, replica_groups=rg,
                ins=[ar_in.opt()], outs=[ar_out.opt()],
            )
            return ar_out

        def squash(ar_src, row_off, final, q):
            """v = squash(s), DVE-only (no ACT/Sync queue head-blocking):
            rsqrt via fast-inverse-sqrt bit trick + 2 Newton steps."""
            for g in range(4):
                nc.gpsimd.dma_start(
                    s_rep[32 * g:32 * g + 32, :],
                    ar_src[row_off:row_off + BC, :],
                )
            nc.scalar.activation(p2, s_rep[:], Act.Square)
            p3 = p2.rearrange("p (d j) -> p j d", d=D_OUT, j=N_OUT)
            nc.vector.reduce_sum(s2[:], p3, axis=Axis.X)
            # 1/sqrt(s2+eps) = exp(-0.5*ln(s2+eps))
            nc.scalar.activation(lns[:], s2[:], Act.Ln, bias=epsc[:])
            nc.scalar.activation(rsq[:], lns[:], Act.Exp, scale=-0.5)
            nc.vector.tensor_scalar_add(den[:], s2[:], 1.0 + EPS)
            nc.vector.reciprocal(rinv[:], den[:])
            nc.vector.tensor_mul(fsc[:], rsq[:], rinv[:])
            nc.vector.tensor_mul(fsc[:], fsc[:], s2[:])
            s3 = s_rep[:].rearrange("p (d j) -> p d j", d=D_OUT, j=N_OUT)
            if not final:
                f3 = fsc[:].unsqueeze(1).to_broadcast((128, D_OUT, N_OUT))
                v3 = v_bf[:].rearrange("p (d j) -> p d j", d=D_OUT, j=N_OUT)
                nc.vector.tensor_tensor(v3, s3, f3, AluOp.mult)
            else:
                # final output, reference layout v[b, j*32+d]
                vf = v_fin[:].rearrange("p (j d) -> p d j", j=N_OUT, d=D_OUT)
                nc.vector.tensor_tensor(
                    vf,
                    s_rep[0:BC, :].rearrange(
                        "p (d j) -> p d j", d=D_OUT, j=N_OUT),
                    fsc[0:BC, :].unsqueeze(1).to_broadcast(
                        (BC, D_OUT, N_OUT)),
                    AluOp.mult)
                nc.gpsimd.dma_start(v_out[q * BC:(q + 1) * BC, :], v_fin[:])

        def rounds(q):
            """Rounds t=1,2 for chunk q; the t=1 agreement was already
            emitted inline with phase 1, and the final squash is deferred to
            the caller (hides the last AllReduce's latency)."""
            ar_out = None
            for t in (1, 2):
                if t == 2:
                    for sec in range(NSEC):
                        agreement_section(t, sec)
                softmax()
                ar_out = weighted_sum(q, t)
                if t < 2:
                    squash(ar_out, 0, False, q)
            return ar_out

        # ------------------------------------------------------ emission
        s0_burst()
        nc.sync.dma_start(xz[:], xz_in[:])   # first needed by phase1 blk 0
        nc.scalar.copy(s0_sb[:], ps_s0[:])
        nc.gpsimd.dma_start(ar0_in[:], s0_sb[:])
        nc.gpsimd.collective_compute(
            "AllReduce", AluOp.add, replica_groups=rg,
            ins=[ar0_in.opt()], outs=[ar0_out.opt()],
        )
        # v0-squash for chunk 0 is emitted inside phase 1 right before the
        # first inlined agreement section, so the DVE drains the early
        # phase-1 psums while the s0 AllReduce is still in flight.
        phase1(0, agree_lag=3,
               pre_agree=lambda: squash(ar0_out, 0, False, 0),
               dve_drain_mod=2)
        ar_last0 = rounds(0)
        squash(ar0_out, BC, False, 1)       # v0 for chunk 1
        # chunk 1 phase 1 overlaps chunk 0's final AllReduce latency
        phase1(1, agree_lag=0)
        squash(ar_last0, 0, True, 0)        # chunk 0 output
        ar_last1 = rounds(1)
        squash(ar_last1, 0, True, 1)

    _fix_sync_waits(nc)
    return nc


# ---------------------------------------------------------------- host prep
def _prep_inputs(x, W):
    """Per-core input maps.

    Local capsule l = blk*8 + g2*4 + gp*2 + h.
    SBUF rows r128 = g2*64 + gp*32 + hp*16 + k.
      wt[r128; blk*1024 + d*32 + j] = W[l(blk,g2,gp,hp), j, d, k]
      xz[r128; blk*128 + h*64 + b]  = x[b, l(blk,g2,gp,h), k] if hp==h else 0
      xd[r128; blk*64 + b]          = x[b, l(blk,g2,gp,hp), k] / 32
    """
    import jax.numpy as jnp

    def tobf(a):
        return np.asarray(jnp.asarray(a).astype(jnp.bfloat16))

    in_maps = []
    ones32 = np.zeros((128, BC), np.float32)
    for p in range(128):
        ones32[p, p % 32] = 1.0
    ones32 = tobf(ones32)
    for c in range(N_CORES):
        xi = x[:, c * I_LOC:(c + 1) * I_LOC, :]          # [B, 256, 16]
        wi = W[0, c * I_LOC:(c + 1) * I_LOC]             # [256, 32, 32, 16]
        # l = blk*8 + g2*4 + gp*2 + h
        x6 = xi.reshape(B, NBLK, 2, 2, 2, D_IN)          # b,blk,g2,gp,h,k
        w7 = wi.reshape(NBLK, 2, 2, 2, N_OUT, D_OUT, D_IN)  # blk,g2,gp,h,j,d,k

        # wt[(g2,gp,hp,k); (blk, d, j)]
        wt = np.transpose(w7, (1, 2, 3, 6, 0, 5, 4)).reshape(128, NBLK * JD)

        # xz[(g2,gp,hp,k); (blk, h, b)] with h-select zero interleave
        xt = np.transpose(x6, (2, 3, 4, 5, 1, 0))        # g2,gp,h,k,blk,b
        xz = np.zeros((2, 2, 2, D_IN, NBLK, 2, B), np.float32)
        for h in range(2):
            xz[:, :, h, :, :, h, :] = xt[:, :, h]
        xz = xz.reshape(128, NBLK * 2 * B)

        # xd[(g2,gp,hp,k); (blk, b)] = x/32 dense
        xd = (xt / 32.0).reshape(128, NBLK * B)

        in_maps.append({
            "wt": tobf(np.ascontiguousarray(wt)),
            "xz": tobf(np.ascontiguousarray(xz)),
            "xd": tobf(np.ascontiguousarray(xd)),
            "ones32": ones32,
        })
    return in_maps


_cached = {}


def _get_program():
    if "nc" not in _cached:
        _cached["nc"] = _build_program()
    return _cached["nc"]


def kernel(x, W):
    x = np.asarray(x, dtype=np.float32)
    W = np.asarray(W, dtype=np.float32)
    nc = _get_program()
    in_maps = _prep_inputs(x, W)
    res = bass_utils.run_bass_kernel_spmd(
        nc, in_maps, core_ids=list(range(N_CORES))
    )
    v = res.results[0]["v"].reshape(B, N_OUT, D_OUT)
    return v.astype(np.float32)


# revision 20
# speedup vs baseline: 1.1116x; 1.1116x over previous
"""Trainium2 Bass kernel for CapsuleLayer dynamic routing (v2).

Problem: x [64, 2048, 16], W [1, 2048, 32, 32, 16] ->
  u_hat = einsum('bik,ijdk->bijd', x, W[0])           [B, N_in, N_out, D_out]
  3 rounds of routing (softmax over j, weighted sum over i, squash),
  returns v [64, 32, 32].

Sharding: N_in (2048) split over 8 cores, 256 input capsules each; per-round
partial weighted sums are AllReduced; squash/softmax replicated.

Per-core design (derived from a perfetto-trace cost model of the v1 kernel,
which was DVE-bound at 81% occupancy):
  * Batch processed in 2 chunks of 32 (u_hat chunk = 16.8MB bf16 in SBUF).
  * U layout [p=(q4, b32); (blk32, g2 2, d32, j32)] with j INNERMOST:
    both big per-round multiplies (U*v-broadcast and U*c-broadcast) hit the
    DVE 2x_1P mode (broadcast strides live on non-innermost dims).
  * All big reductions are TT-add TREES at 2x instead of native
    tensor_reduce (which is capped at 1x).
  * Phase 1 computes u_hat with 8-way 32x32 PE-array tile packing
    (row bands (g2,gp) x col bands (gp,h)), sharing one W stream per blk
    with the K=128 full-array s0 matmuls (t=0 shortcut, keeps HAM warm).
  * PSUM->SBUF drains split between DVE (CAST 2x) and ACT.
  * Final (sec, q) reduction of weighted-sum partials via a ones-matmul on
    the PE with psum accumulation.
  * squash uses only exp/ln (one ACT table set): rsqrt(x) = exp(-0.5*ln(x)).
"""
import sys

sys.path.insert(0, '/opt/trn_rl_repo')

import numpy as np

import concourse.bass as bass
import concourse.mybir as mybir
from concourse import bass_utils, tile

# ---------------------------------------------------------------- constants
N_CORES = 8
B = 64
N_IN = 2048
D_IN = 16
N_OUT = 32
D_OUT = 32
ROUTINGS = 3
EPS = 1e-9

I_LOC = N_IN // N_CORES          # 256 local capsules
NBLK = I_LOC // 8                # 32 blocks of 8 capsules
BC = 32                          # batch chunk
NCHUNK = B // BC                 # 2
JD = N_OUT * D_OUT               # 1024 (d,j) values per capsule
NSEC = 8                         # routing sections per chunk-round
SEC_BLKS = NBLK // NSEC          # 4 blocks per section
SEC_BH = SEC_BLKS * 2            # 8 (blk, g2) groups per section
SEC_COLS = SEC_BH * JD           # 8192 U-columns per section

f32 = mybir.dt.float32
bf16 = mybir.dt.bfloat16

_MAX_WAITS = 1
_carrier = [0]


def _patch_tile():
    """Work around this walrus build rejecting >1 sync wait per instruction."""
    import concourse.mybir as _mybir
    from concourse import tile as _tile
    from concourse.tile import TileContext as _TC

    def _drain_and_barrier(self, tick_clock, wait_clock):
        ScopedClock = _tile.ScopedClock
        probe = self.nc.sync.nop(nofuse=True)
        wait_clock.add_sem_waits(
            probe.ins, ScopedClock({None: tick_clock.global_clock})
        )
        si = probe.ins.sync_info
        waits = list(si.on_wait)
        probe.ins.sync_info = _mybir.SyncInfo(
            on_wait=waits[:1], on_update=list(si.on_update)
        )
        for w in waits[1:]:
            carrier = self.nc.sync.nop(nofuse=True)
            carrier.ins.sync_info = _mybir.SyncInfo(on_wait=[w], on_update=[])
        self.nc.sync.drain()
        self.nc.all_engine_barrier()
        assert self.sems is not None
        popped = self.nc._tile_sem_poison_stack.pop()
        assert popped is self._sem_poison
        self.nc.clear_and_free_semaphores(list(self.sems.allocated().values()))
        self.nc.all_engine_barrier()

    _TC._drain_and_barrier = _drain_and_barrier

    try:
        from concourse import tile_utils
        tile_utils.max_sbuf_usage = 208 * 1024
    except Exception:
        pass


def _fix_sync_waits(nc, max_waits=_MAX_WAITS):
    n_fixed = 0
    for func in nc.m.functions:
        for bb in func.blocks:
            insts = list(bb.instructions)
            new_list = []
            changed = False
            for inst in insts:
                si = getattr(inst, "sync_info", None)
                waits = list(si.on_wait) if si is not None else []
                if len(waits) > max_waits:
                    keep = waits[: max_waits - 1] if max_waits > 1 else []
                    hoist = waits[len(keep):-1]
                    tail = [waits[-1]]
                    for w in hoist:
                        _carrier[0] += 1
                        nop = mybir.InstNoOp(
                            name=f"syncfix-{_carrier[0]}", engine=inst.engine
                        )
                        nop.sync_info = mybir.SyncInfo(on_wait=[w], on_update=[])
                        new_list.append(nop)
                    inst.sync_info = mybir.SyncInfo(
                        on_wait=keep + tail, on_update=list(si.on_update)
                    )
                    changed = True
                    n_fixed += 1
                new_list.append(inst)
            if changed:
                bb.instructions = new_list
    return n_fixed


# ---------------------------------------------------------------- program
def _build_program():
    _patch_tile()
    nc = bass.Bass(trn_type="TRN2", num_devices=N_CORES)

    wt_in = nc.dram_tensor("wt", [128, NBLK * JD], bf16, kind="ExternalInput")
    xz_in = nc.dram_tensor("xz", [128, NBLK * 2 * B], bf16, kind="ExternalInput")
    xd_in = nc.dram_tensor("xd", [128, NBLK * B], bf16, kind="ExternalInput")
    ones_in = nc.dram_tensor("ones32", [128, BC], bf16, kind="ExternalInput")
    v_out = nc.dram_tensor("v", [B, JD], f32, kind="ExternalOutput")

    AluOp = mybir.AluOpType
    Act = mybir.ActivationFunctionType
    Axis = mybir.AxisListType
    rg = [list(range(N_CORES))]

    from contextlib import ExitStack
    with tile.TileContext(nc, num_cores=N_CORES) as tc, ExitStack() as es:
        cpool = es.enter_context(tc.tile_pool(name="const", bufs=1))
        wpool = es.enter_context(tc.tile_pool(name="wstream", bufs=5))
        upool = es.enter_context(tc.tile_pool(name="ubuf", bufs=1))
        scpool = es.enter_context(tc.tile_pool(name="scratch", bufs=1))
        smpool = es.enter_context(tc.tile_pool(name="small", bufs=1))
        ps1pool = es.enter_context(tc.tile_pool(name="psph1", bufs=2, space="PSUM"))
        ps0pool = es.enter_context(tc.tile_pool(name="psums0", bufs=1, space="PSUM"))
        psrpool = es.enter_context(tc.tile_pool(name="psred", bufs=1, space="PSUM"))
        dpool = es.enter_context(tc.tile_pool(name="dram", bufs=1, space="DRAM"))

        # ---- constants / inputs resident in SBUF
        epsc = cpool.tile([128, 1], f32, tag="epsc")
        nc.vector.memset(epsc[:], EPS)
        xz = cpool.tile([128, NBLK * 2 * B], bf16)
        xd = cpool.tile([128, NBLK * B], bf16)
        ones32 = cpool.tile([128, BC], bf16)
        nc.sync.dma_start(xd[:], xd_in[:])
        nc.sync.dma_start(ones32[:], ones_in[:])

        # ---- big buffers
        U = upool.tile([128, NBLK * 2 * JD], bf16, tag="U")       # 128KB/p
        prodb = scpool.tile([128, SEC_COLS], bf16, tag="prod")    # 16KB/p
        treeb = scpool.tile([128, SEC_COLS // 2], bf16, tag="tree")  # 8KB/p
        algA = smpool.tile([128, NBLK * 2 * N_OUT], f32, tag="algA")  # 8KB/p
        cbuf = smpool.tile([128, NBLK * 2 * N_OUT], bf16, tag="c")
        Zt = smpool.tile([128, NBLK * 2], f32, tag="Z")
        Zr = smpool.tile([128, NBLK * 2], f32, tag="Zr")
        s_rep = smpool.tile([128, JD], f32, tag="srep")
        s2 = smpool.tile([128, N_OUT], f32, tag="s2")
        lns = smpool.tile([128, N_OUT], f32, tag="lns")
        rsq = smpool.tile([128, N_OUT], f32, tag="rsq")
        den = smpool.tile([128, N_OUT], f32, tag="den")
        rinv = smpool.tile([128, N_OUT], f32, tag="rinv")
        fsc = smpool.tile([128, N_OUT], f32, tag="fsc")
        ibuf = smpool.tile([128, N_OUT], mybir.dt.int32, tag="ibuf")
        v_bf = smpool.tile([128, JD], bf16, tag="vbf")
        s_sb = smpool.tile([BC, JD], f32, tag="ssb")
        s0_sb = smpool.tile([B, JD], f32, tag="s0sb")
        v_fin = smpool.tile([BC, JD], f32, tag="vfin")

        # aliased scratch views (prodb/treeb free at those times)
        ebuf = prodb[:, :4096].bitcast(f32)          # [128, 2048] exp(logits)
        p2 = prodb[:, 4096:6144].bitcast(f32)        # [128, 1024] squash s^2
        tsec = treeb[:, 2048:2560].bitcast(f32)      # [128, 256] t=2 A-slice

        ps_s0 = ps0pool.tile([B, JD], f32, tag="pss0")

        # AR dram staging
        ar0_in = dpool.tile([B, JD], f32, tag="ar0i")
        ar0_out = dpool.tile([B, JD], f32, tag="ar0o")
        ar_bufs = {}
        for q in range(NCHUNK):
            for t in (1, 2):
                ar_bufs[(q, t)] = (
                    dpool.tile([BC, JD], f32, name=f"ari{q}{t}"),
                    dpool.tile([BC, JD], f32, name=f"aro{q}{t}"),
                )

        # ------------------------------------------------------ phase 1
        def warmup():
            """~6us of back-to-back full-array matmuls to flip the PE HAM
            clock gate to 8/8 before a phase-1 burst (output never read)."""
            ps = ps1pool.tile([128, JD], f32, tag="ps1", name="warm")
            for i in range(14):
                nc.tensor.matmul(
                    ps[0:64, 0:512], xd[:, 0:B], xz[:, 0:512],
                    start=True, stop=True,
                )

        def s0_burst():
            """t=0 shortcut: s0 = (1/32) sum_i u_hat, full batch, K=128
            full-array matmuls, own W stream; runs before phase 1 so the
            AllReduce of s0 completes early (also warms the PE HAM)."""
            for blk in range(NBLK):
                w = wpool.tile([128, JD], bf16, tag="w", name="w0")
                nc.sync.dma_start(w[:], wt_in[:, blk * JD:(blk + 1) * JD])
                for half in range(2):
                    nc.tensor.matmul(
                        ps_s0[:, half * 512:(half + 1) * 512],
                        xd[:, blk * B:(blk + 1) * B],
                        w[:, half * 512:(half + 1) * 512],
                        start=(blk == 0), stop=(blk == NBLK - 1),
                    )

        def phase1(q, agree_lag=None, pre_agree=None, dve_drain_mod=0):
            """u_hat for batch chunk q into U.  If agree_lag is not None,
            the round-1 agreement sections are emitted inline, lagged by
            `agree_lag` phase-1 sections, to fill the DVE idle window while
            the PE streams the tiled matmuls.  pre_agree() is emitted right
            before the first inlined agreement section.  A drain goes to the
            DVE when (idx % 4) < dve_drain_mod, else to ACT."""
            emitted_pre = [False]
            for blk in range(NBLK):
                w = wpool.tile([128, JD], bf16, tag="w")
                nc.sync.dma_start(w[:], wt_in[:, blk * JD:(blk + 1) * JD])
                for g2 in range(2):
                    ps = ps1pool.tile([128, JD], f32, tag="ps1")
                    for gp in range(2):
                        r = g2 * 2 + gp
                        for h in range(2):
                            m = gp * 2 + h
                            lhs = xz[32 * r:32 * r + 32,
                                     blk * 2 * B + h * B + q * BC:
                                     blk * 2 * B + h * B + q * BC + BC]
                            for half in range(2):
                                nc.tensor.matmul(
                                    ps[32 * m:32 * m + 32,
                                       half * 512:(half + 1) * 512],
                                    lhs,
                                    w[32 * r:32 * r + 32,
                                      half * 512:(half + 1) * 512],
                                    start=True, stop=True,
                                    tile_position=(32 * r, 32 * m),
                                )
                    dst = U[:, blk * 2 * JD + g2 * JD:
                            blk * 2 * JD + (g2 + 1) * JD]
                    if (blk * 2 + g2) % 4 < dve_drain_mod:
                        nc.vector.tensor_copy(dst, ps[:])
                    else:
                        nc.scalar.copy(dst, ps[:])
                if agree_lag is not None and (blk + 1) % SEC_BLKS == 0:
                    p1sec = (blk + 1) // SEC_BLKS - 1
                    asec = p1sec - agree_lag
                    if asec >= 0:
                        if not emitted_pre[0]:
                            emitted_pre[0] = True
                            if pre_agree is not None:
                                pre_agree()
                        agreement_section(1, asec)
            if agree_lag is not None:
                for asec in range(NSEC - agree_lag, NSEC):
                    agreement_section(1, asec)

        # ------------------------------------------------------ routing ops
        def agreement_section(t, sec):
                Us = U[:, sec * SEC_COLS:(sec + 1) * SEC_COLS].rearrange(
                    "p (bh d j) -> p bh d j", bh=SEC_BH, d=D_OUT, j=N_OUT)
                P = prodb[:].rearrange(
                    "p (bh d j) -> p bh d j", bh=SEC_BH, d=D_OUT, j=N_OUT)
                v4 = (v_bf[:]
                      .rearrange("p (d j) -> p d j", d=D_OUT, j=N_OUT)
                      .unsqueeze(1)
                      .to_broadcast((128, SEC_BH, D_OUT, N_OUT)))
                nc.vector.tensor_tensor(P, Us, v4, AluOp.mult)   # 2x
                # d-tree: 32 -> 16 -> 8 -> 4 -> 2 -> 1
                T1 = treeb[:, :4096].rearrange(
                    "p (bh d j) -> p bh d j", bh=SEC_BH, d=16, j=N_OUT)
                nc.vector.tensor_tensor(
                    T1, P[:, :, 0:16, :], P[:, :, 16:32, :], AluOp.add)
                T2 = prodb[:, :2048].rearrange(
                    "p (bh d j) -> p bh d j", bh=SEC_BH, d=8, j=N_OUT)
                nc.vector.tensor_tensor(
                    T2, T1[:, :, 0:8, :], T1[:, :, 8:16, :], AluOp.add)
                T3 = treeb[:, :1024].rearrange(
                    "p (bh d j) -> p bh d j", bh=SEC_BH, d=4, j=N_OUT)
                nc.vector.tensor_tensor(
                    T3, T2[:, :, 0:4, :], T2[:, :, 4:8, :], AluOp.add)
                T4 = prodb[:, 2048:2560].rearrange(
                    "p (bh d j) -> p bh d j", bh=SEC_BH, d=2, j=N_OUT)
                nc.vector.tensor_tensor(
                    T4, T3[:, :, 0:2, :], T3[:, :, 2:4, :], AluOp.add)
                a_cols = SEC_BH * N_OUT            # 256 logit cols / section
                if t == 1:
                    T5 = algA[:, sec * a_cols:(sec + 1) * a_cols].rearrange(
                        "p (bh d j) -> p bh d j", bh=SEC_BH, d=1, j=N_OUT)
                    nc.vector.tensor_tensor(
                        T5, T4[:, :, 0:1, :], T4[:, :, 1:2, :], AluOp.add)
                else:
                    T5 = tsec.rearrange(
                        "p (bh d j) -> p bh d j", bh=SEC_BH, d=1, j=N_OUT)
                    nc.vector.tensor_tensor(
                        T5, T4[:, :, 0:1, :], T4[:, :, 1:2, :], AluOp.add)
                    asl = algA[:, sec * a_cols:(sec + 1) * a_cols]
                    nc.vector.tensor_add(asl, asl, tsec)

        def softmax():
            nc.scalar.activation(ebuf, algA[:], Act.Exp)
            e3 = ebuf.rearrange("p (bh j) -> p bh j", bh=NBLK * 2, j=N_OUT)
            nc.vector.reduce_sum(Zt[:], e3, axis=Axis.X)
            nc.vector.reciprocal(Zr[:], Zt[:])
            c3 = cbuf[:].rearrange("p (bh j) -> p bh j", bh=NBLK * 2, j=N_OUT)
            zr3 = Zr[:].unsqueeze(2).to_broadcast((128, NBLK * 2, N_OUT))
            nc.vector.tensor_tensor(c3, e3, zr3, AluOp.mult)

        def weighted_sum(q, t):
            """s_partial = sum_i c * U.  The DVE only computes the c*U
            product (2x); ALL the summation (64 bh column-groups AND the 4
            partition groups) happens on the PE as one 128-matmul psum
            accumulation chain through the ones matrix."""
            ps_red = psrpool.tile([BC, JD], f32, tag="psred")
            NSW = 16                       # 2-blk product sections
            SW_BH = NBLK * 2 // NSW        # 4 (blk,g2) groups per section
            SW_COLS = SW_BH * JD           # 4096
            slots = [prodb[:, 0:SW_COLS], prodb[:, SW_COLS:2 * SW_COLS],
                     treeb[:, 0:SW_COLS]]
            for sec in range(NSW):
                slot = slots[sec % 3]
                P = slot.rearrange(
                    "p (bh d j) -> p bh d j", bh=SW_BH, d=D_OUT, j=N_OUT)
                Us = U[:, sec * SW_COLS:(sec + 1) * SW_COLS].rearrange(
                    "p (bh d j) -> p bh d j", bh=SW_BH, d=D_OUT, j=N_OUT)
                a_cols = SW_BH * N_OUT
                c4 = (cbuf[:, sec * a_cols:(sec + 1) * a_cols]
                      .rearrange("p (bh j) -> p bh j", bh=SW_BH, j=N_OUT)
                      .unsqueeze(2)
                      .to_broadcast((128, SW_BH, D_OUT, N_OUT)))
                nc.vector.tensor_tensor(P, Us, c4, AluOp.mult)   # 2x
                for bh in range(SW_BH):
                    for half in range(2):
                        nc.tensor.matmul(
                            ps_red[:, half * 512:(half + 1) * 512],
                            ones32[:],
                            slot[:, bh * JD + half * 512:
                                 bh * JD + half * 512 + 512],
                            start=(sec == 0 and bh == 0),
                            stop=(sec == NSW - 1 and bh == SW_BH - 1),
                        )
            nc.scalar.copy(s_sb[:], ps_red[:])
            ar_in, ar_out = ar_bufs[(q, t)]
            nc.gpsimd.dma_start(ar_in[:], s_sb[:])
            nc.gpsimd.collective_compute(
                "AllReduce", AluOp.add, replica_groups=rg,
                ins=[ar_in.opt()], outs=[ar_out.opt()],
            )
            return ar_out

        def squash(ar_src, row_off, final, q):
            """v = squash(s), DVE-only (no ACT/Sync queue head-blocking):
            rsqrt via fast-inverse-sqrt bit trick + 2 Newton steps."""
            for g in range(4):
                nc.gpsimd.dma_start(
                    s_rep[32 * g:32 * g + 32, :],
                    ar_src[row_off:row_off + BC, :],
                )
            nc.vector.tensor_mul(p2, s_rep[:], s_rep[:])
            p3 = p2.rearrange("p (d j) -> p j d", d=D_OUT, j=N_OUT)
            nc.vector.reduce_sum(s2[:], p3, axis=Axis.X)
            nc.vector.tensor_scalar_add(den[:], s2[:], 1.0 + EPS)
            nc.vector.tensor_scalar_add(lns[:], s2[:], EPS)   # x = s2+eps
            ii = lns[:].bitcast(mybir.dt.int32)
            # y0 = bits(0x5f3759df - (bits(x) >> 1))
            nc.vector.tensor_scalar(
                ibuf[:], ii, 1, None,
                mybir.AluOpType.logical_shift_right)
            nc.vector.tensor_scalar(
                ibuf[:], ibuf[:], 0x5F3759DF, -1,
                mybir.AluOpType.subtract, mybir.AluOpType.mult)
            y0 = ibuf[:].bitcast(f32)
            # two Newton steps: y <- y*(1.5 - 0.5*x*y^2)
            nc.vector.tensor_mul(rsq[:], y0, y0)
            nc.vector.tensor_mul(rsq[:], rsq[:], lns[:])
            nc.vector.tensor_scalar(
                rsq[:], rsq[:], -0.5, 1.5,
                mybir.AluOpType.mult, mybir.AluOpType.add)
            nc.vector.tensor_mul(rsq[:], rsq[:], y0)
            nc.vector.tensor_mul(fsc[:], rsq[:], rsq[:])
            nc.vector.tensor_mul(fsc[:], fsc[:], lns[:])
            nc.vector.tensor_scalar(
                fsc[:], fsc[:], -0.5, 1.5,
                mybir.AluOpType.mult, mybir.AluOpType.add)
            nc.vector.tensor_mul(rsq[:], rsq[:], fsc[:])
            nc.vector.reciprocal(rinv[:], den[:])
            nc.vector.tensor_mul(fsc[:], rsq[:], rinv[:])
            nc.vector.tensor_mul(fsc[:], fsc[:], s2[:])
            s3 = s_rep[:].rearrange("p (d j) -> p d j", d=D_OUT, j=N_OUT)
            if not final:
                f3 = fsc[:].unsqueeze(1).to_broadcast((128, D_OUT, N_OUT))
                v3 = v_bf[:].rearrange("p (d j) -> p d j", d=D_OUT, j=N_OUT)
                nc.vector.tensor_tensor(v3, s3, f3, AluOp.mult)
            else:
                # final output, reference layout v[b, j*32+d]
                vf = v_fin[:].rearrange("p (j d) -> p d j", j=N_OUT, d=D_OUT)
                nc.vector.tensor_tensor(
                    vf,
                    s_rep[0:BC, :].rearrange(
                        "p (d j) -> p d j", d=D_OUT, j=N_OUT),
                    fsc[0:BC, :].unsqueeze(1).to_broadcast(
                        (BC, D_OUT, N_OUT)),
                    AluOp.mult)
                nc.gpsimd.dma_start(v_out[q * BC:(q + 1) * BC, :], v_fin[:])

        def rounds(q):
            """Rounds t=1,2 for chunk q; the t=1 agreement was already
            emitted inline with phase 1, and the final squash is deferred to
            the caller (hides the last AllReduce's latency)."""
            ar_out = None
            for t in (1, 2):
                if t == 2:
                    for sec in range(NSEC):
                        agreement_section(t, sec)
                softmax()
                ar_out = weighted_sum(q, t)
                if t < 2:
                    squash(ar_out, 0, False, q)
            return ar_out

        # ------------------------------------------------------ emission
        s0_burst()
        nc.sync.dma_start(xz[:], xz_in[:])   # first needed by phase1 blk 0
        nc.scalar.copy(s0_sb[:], ps_s0[:])
        nc.gpsimd.dma_start(ar0_in[:], s0_sb[:])
        nc.gpsimd.collective_compute(
            "AllReduce", AluOp.add, replica_groups=rg,
            ins=[ar0_in.opt()], outs=[ar0_out.opt()],
        )
        # v0-squash for chunk 0 is emitted inside phase 1 right before the
        # first inlined agreement section, so the DVE drains the early
        # phase-1 psums while the s0 AllReduce is still in flight.
        phase1(0, agree_lag=3,
               pre_agree=lambda: squash(ar0_out, 0, False, 0),
               dve_drain_mod=1)
        ar_last0 = rounds(0)
        squash(ar0_out, BC, False, 1)       # v0 for chunk 1
        # chunk 1 phase 1 overlaps chunk 0's final AllReduce latency
        phase1(1, agree_lag=0)
        squash(ar_last0, 0, True, 0)        # chunk 0 output
        ar_last1 = rounds(1)
        squash(ar_last1, 0, True, 1)

    _fix_sync_waits(nc)
    return nc


# ---------------------------------------------------------------- host prep
def _prep_inputs(x, W):
    """Per-core input maps.

    Local capsule l = blk*8 + g2*4 + gp*2 + h.
    SBUF rows r128 = g2*64 + gp*32 + hp*16 + k.
      wt[r128; blk*1024 + d*32 + j] = W[l(blk,g2,gp,hp), j, d, k]
      xz[r128; blk*128 + h*64 + b]  = x[b, l(blk,g2,gp,h), k] if hp==h else 0
      xd[r128; blk*64 + b]          = x[b, l(blk,g2,gp,hp), k] / 32
    """
    import jax.numpy as jnp

    def tobf(a):
        return np.asarray(jnp.asarray(a).astype(jnp.bfloat16))

    in_maps = []
    ones32 = np.zeros((128, BC), np.float32)
    for p in range(128):
        ones32[p, p % 32] = 1.0
    ones32 = tobf(ones32)
    for c in range(N_CORES):
        xi = x[:, c * I_LOC:(c + 1) * I_LOC, :]          # [B, 256, 16]
        wi = W[0, c * I_LOC:(c + 1) * I_LOC]             # [256, 32, 32, 16]
        # l = blk*8 + g2*4 + gp*2 + h
        x6 = xi.reshape(B, NBLK, 2, 2, 2, D_IN)          # b,blk,g2,gp,h,k
        w7 = wi.reshape(NBLK, 2, 2, 2, N_OUT, D_OUT, D_IN)  # blk,g2,gp,h,j,d,k

        # wt[(g2,gp,hp,k); (blk, d, j)]
        wt = np.transpose(w7, (1, 2, 3, 6, 0, 5, 4)).reshape(128, NBLK * JD)

        # xz[(g2,gp,hp,k); (blk, h, b)] with h-select zero interleave
        xt = np.transpose(x6, (2, 3, 4, 5, 1, 0))        # g2,gp,h,k,blk,b
        xz = np.zeros((2, 2, 2, D_IN, NBLK, 2, B), np.float32)
        for h in range(2):
            xz[:, :, h, :, :, h, :] = xt[:, :, h]
        xz = xz.reshape(128, NBLK * 2 * B)

        # xd[(g2,gp,hp,k); (blk, b)] = x/32 dense
        xd = (xt / 32.0).reshape(128, NBLK * B)

        in_maps.append({
            "wt": tobf(np.ascontiguousarray(wt)),
            "xz": tobf(np.ascontiguousarray(xz)),
            "xd": tobf(np.ascontiguousarray(xd)),
            "ones32": ones32,
        })
    return in_maps


_cached = {}


def _get_program():
    if "nc" not in _cached:
        _cached["nc"] = _build_program()
    return _cached["nc"]


def kernel(x, W):
    x = np.asarray(x, dtype=np.float32)
    W = np.asarray(W, dtype=np.float32)
    nc = _get_program()
    in_maps = _prep_inputs(x, W)
    res = bass_utils.run_bass_kernel_spmd(
        nc, in_maps, core_ids=list(range(N_CORES))
    )
    v = res.results[0]["v"].reshape(B, N_OUT, D_OUT)
    return v.astype(np.float32)


# revision 21
# speedup vs baseline: 1.1257x; 1.0127x over previous
"""Trainium2 Bass kernel for CapsuleLayer dynamic routing (v2).

Problem: x [64, 2048, 16], W [1, 2048, 32, 32, 16] ->
  u_hat = einsum('bik,ijdk->bijd', x, W[0])           [B, N_in, N_out, D_out]
  3 rounds of routing (softmax over j, weighted sum over i, squash),
  returns v [64, 32, 32].

Sharding: N_in (2048) split over 8 cores, 256 input capsules each; per-round
partial weighted sums are AllReduced; squash/softmax replicated.

Per-core design (derived from a perfetto-trace cost model of the v1 kernel,
which was DVE-bound at 81% occupancy):
  * Batch processed in 2 chunks of 32 (u_hat chunk = 16.8MB bf16 in SBUF).
  * U layout [p=(q4, b32); (blk32, g2 2, d32, j32)] with j INNERMOST:
    both big per-round multiplies (U*v-broadcast and U*c-broadcast) hit the
    DVE 2x_1P mode (broadcast strides live on non-innermost dims).
  * All big reductions are TT-add TREES at 2x instead of native
    tensor_reduce (which is capped at 1x).
  * Phase 1 computes u_hat with 8-way 32x32 PE-array tile packing
    (row bands (g2,gp) x col bands (gp,h)), sharing one W stream per blk
    with the K=128 full-array s0 matmuls (t=0 shortcut, keeps HAM warm).
  * PSUM->SBUF drains split between DVE (CAST 2x) and ACT.
  * Final (sec, q) reduction of weighted-sum partials via a ones-matmul on
    the PE with psum accumulation.
  * squash uses only exp/ln (one ACT table set): rsqrt(x) = exp(-0.5*ln(x)).
"""
import sys

sys.path.insert(0, '/opt/trn_rl_repo')

import numpy as np

import concourse.bass as bass
import concourse.mybir as mybir
from concourse import bass_utils, tile

# ---------------------------------------------------------------- constants
N_CORES = 8
B = 64
N_IN = 2048
D_IN = 16
N_OUT = 32
D_OUT = 32
ROUTINGS = 3
EPS = 1e-9

I_LOC = N_IN // N_CORES          # 256 local capsules
NBLK = I_LOC // 8                # 32 blocks of 8 capsules
BC = 32                          # batch chunk
NCHUNK = B // BC                 # 2
JD = N_OUT * D_OUT               # 1024 (d,j) values per capsule
NSEC = 8                         # routing sections per chunk-round
SEC_BLKS = NBLK // NSEC          # 4 blocks per section
SEC_BH = SEC_BLKS * 2            # 8 (blk, g2) groups per section
SEC_COLS = SEC_BH * JD           # 8192 U-columns per section

f32 = mybir.dt.float32
bf16 = mybir.dt.bfloat16

_MAX_WAITS = 1
_carrier = [0]


def _patch_tile():
    """Work around this walrus build rejecting >1 sync wait per instruction."""
    import concourse.mybir as _mybir
    from concourse import tile as _tile
    from concourse.tile import TileContext as _TC

    def _drain_and_barrier(self, tick_clock, wait_clock):
        ScopedClock = _tile.ScopedClock
        probe = self.nc.sync.nop(nofuse=True)
        wait_clock.add_sem_waits(
            probe.ins, ScopedClock({None: tick_clock.global_clock})
        )
        si = probe.ins.sync_info
        waits = list(si.on_wait)
        probe.ins.sync_info = _mybir.SyncInfo(
            on_wait=waits[:1], on_update=list(si.on_update)
        )
        for w in waits[1:]:
            carrier = self.nc.sync.nop(nofuse=True)
            carrier.ins.sync_info = _mybir.SyncInfo(on_wait=[w], on_update=[])
        self.nc.sync.drain()
        self.nc.all_engine_barrier()
        assert self.sems is not None
        popped = self.nc._tile_sem_poison_stack.pop()
        assert popped is self._sem_poison
        self.nc.clear_and_free_semaphores(list(self.sems.allocated().values()))
        self.nc.all_engine_barrier()

    _TC._drain_and_barrier = _drain_and_barrier

    try:
        from concourse import tile_utils
        tile_utils.max_sbuf_usage = 208 * 1024
    except Exception:
        pass


def _fix_sync_waits(nc, max_waits=_MAX_WAITS):
    n_fixed = 0
    for func in nc.m.functions:
        for bb in func.blocks:
            insts = list(bb.instructions)
            new_list = []
            changed = False
            for inst in insts:
                si = getattr(inst, "sync_info", None)
                waits = list(si.on_wait) if si is not None else []
                if len(waits) > max_waits:
                    keep = waits[: max_waits - 1] if max_waits > 1 else []
                    hoist = waits[len(keep):-1]
                    tail = [waits[-1]]
                    for w in hoist:
                        _carrier[0] += 1
                        nop = mybir.InstNoOp(
                            name=f"syncfix-{_carrier[0]}", engine=inst.engine
                        )
                        nop.sync_info = mybir.SyncInfo(on_wait=[w], on_update=[])
                        new_list.append(nop)
                    inst.sync_info = mybir.SyncInfo(
                        on_wait=keep + tail, on_update=list(si.on_update)
                    )
                    changed = True
                    n_fixed += 1
                new_list.append(inst)
            if changed:
                bb.instructions = new_list
    return n_fixed


# ---------------------------------------------------------------- program
def _build_program():
    _patch_tile()
    nc = bass.Bass(trn_type="TRN2", num_devices=N_CORES)

    wt_in = nc.dram_tensor("wt", [128, NBLK * JD], bf16, kind="ExternalInput")
    xz_in = nc.dram_tensor("xz", [128, NBLK * 2 * B], bf16, kind="ExternalInput")
    xd_in = nc.dram_tensor("xd", [128, NBLK * B], bf16, kind="ExternalInput")
    ones_in = nc.dram_tensor("ones32", [128, BC], bf16, kind="ExternalInput")
    v_out = nc.dram_tensor("v", [B, JD], f32, kind="ExternalOutput")

    AluOp = mybir.AluOpType
    Act = mybir.ActivationFunctionType
    Axis = mybir.AxisListType
    rg = [list(range(N_CORES))]

    from contextlib import ExitStack
    with tile.TileContext(nc, num_cores=N_CORES) as tc, ExitStack() as es:
        cpool = es.enter_context(tc.tile_pool(name="const", bufs=1))
        wpool = es.enter_context(tc.tile_pool(name="wstream", bufs=5))
        upool = es.enter_context(tc.tile_pool(name="ubuf", bufs=1))
        scpool = es.enter_context(tc.tile_pool(name="scratch", bufs=1))
        smpool = es.enter_context(tc.tile_pool(name="small", bufs=1))
        ps1pool = es.enter_context(tc.tile_pool(name="psph1", bufs=2, space="PSUM"))
        ps0pool = es.enter_context(tc.tile_pool(name="psums0", bufs=1, space="PSUM"))
        psrpool = es.enter_context(tc.tile_pool(name="psred", bufs=1, space="PSUM"))
        dpool = es.enter_context(tc.tile_pool(name="dram", bufs=1, space="DRAM"))

        # ---- constants / inputs resident in SBUF
        epsc = cpool.tile([128, 1], f32, tag="epsc")
        nc.vector.memset(epsc[:], EPS)
        xz = cpool.tile([128, NBLK * 2 * B], bf16)
        xd = cpool.tile([128, NBLK * B], bf16)
        ones32 = cpool.tile([128, BC], bf16)
        nc.sync.dma_start(xd[:], xd_in[:])
        nc.sync.dma_start(ones32[:], ones_in[:])

        # ---- big buffers
        U = upool.tile([128, NBLK * 2 * JD], bf16, tag="U")       # 128KB/p
        prodb = scpool.tile([128, SEC_COLS], bf16, tag="prod")    # 16KB/p
        treeb = scpool.tile([128, SEC_COLS // 2], bf16, tag="tree")  # 8KB/p
        algA = smpool.tile([128, NBLK * 2 * N_OUT], f32, tag="algA")  # 8KB/p
        cbuf = smpool.tile([128, NBLK * 2 * N_OUT], bf16, tag="c")
        Zt = smpool.tile([128, NBLK * 2], f32, tag="Z")
        Zr = smpool.tile([128, NBLK * 2], f32, tag="Zr")
        s_rep = smpool.tile([128, JD], f32, tag="srep")
        s2 = smpool.tile([128, N_OUT], f32, tag="s2")
        lns = smpool.tile([128, N_OUT], f32, tag="lns")
        rsq = smpool.tile([128, N_OUT], f32, tag="rsq")
        den = smpool.tile([128, N_OUT], f32, tag="den")
        rinv = smpool.tile([128, N_OUT], f32, tag="rinv")
        fsc = smpool.tile([128, N_OUT], f32, tag="fsc")
        ibuf = smpool.tile([128, N_OUT], mybir.dt.int32, tag="ibuf")
        v_bf = smpool.tile([128, JD], bf16, tag="vbf")
        s_sb = smpool.tile([BC, JD], f32, tag="ssb")
        s0_sb = smpool.tile([B, JD], f32, tag="s0sb")
        v_fin = smpool.tile([BC, JD], f32, tag="vfin")

        # aliased scratch views (prodb/treeb free at those times)
        ebuf = prodb[:, :4096].bitcast(f32)          # [128, 2048] exp(logits)
        p2 = prodb[:, 4096:6144].bitcast(f32)        # [128, 1024] squash s^2
        tsec = treeb[:, 2048:2560].bitcast(f32)      # [128, 256] t=2 A-slice

        ps_s0 = ps0pool.tile([B, JD], f32, tag="pss0")

        # AR dram staging
        ar0_in = dpool.tile([B, JD], f32, tag="ar0i")
        ar0_out = dpool.tile([B, JD], f32, tag="ar0o")
        ar_bufs = {}
        for q in range(NCHUNK):
            for t in (1, 2):
                ar_bufs[(q, t)] = (
                    dpool.tile([BC, JD], f32, name=f"ari{q}{t}"),
                    dpool.tile([BC, JD], f32, name=f"aro{q}{t}"),
                )

        # ------------------------------------------------------ phase 1
        def warmup():
            """~6us of back-to-back full-array matmuls to flip the PE HAM
            clock gate to 8/8 before a phase-1 burst (output never read)."""
            ps = ps1pool.tile([128, JD], f32, tag="ps1", name="warm")
            for i in range(14):
                nc.tensor.matmul(
                    ps[0:64, 0:512], xd[:, 0:B], xz[:, 0:512],
                    start=True, stop=True,
                )

        def s0_burst():
            """t=0 shortcut: s0 = (1/32) sum_i u_hat, full batch, K=128
            full-array matmuls, own W stream; runs before phase 1 so the
            AllReduce of s0 completes early (also warms the PE HAM)."""
            for blk in range(NBLK):
                w = wpool.tile([128, JD], bf16, tag="w", name="w0")
                nc.sync.dma_start(w[:], wt_in[:, blk * JD:(blk + 1) * JD])
                for half in range(2):
                    nc.tensor.matmul(
                        ps_s0[:, half * 512:(half + 1) * 512],
                        xd[:, blk * B:(blk + 1) * B],
                        w[:, half * 512:(half + 1) * 512],
                        start=(blk == 0), stop=(blk == NBLK - 1),
                    )

        def phase1(q, agree_lag=None, pre_agree=None, dve_drain_mod=0):
            """u_hat for batch chunk q into U.  If agree_lag is not None,
            the round-1 agreement sections are emitted inline, lagged by
            `agree_lag` phase-1 sections, to fill the DVE idle window while
            the PE streams the tiled matmuls.  pre_agree() is emitted right
            before the first inlined agreement section.  A drain goes to the
            DVE when (idx % 4) < dve_drain_mod, else to ACT."""
            emitted_pre = [False]
            for blk in range(NBLK):
                w = wpool.tile([128, JD], bf16, tag="w")
                nc.sync.dma_start(w[:], wt_in[:, blk * JD:(blk + 1) * JD])
                for g2 in range(2):
                    ps = ps1pool.tile([128, JD], f32, tag="ps1")
                    for gp in range(2):
                        r = g2 * 2 + gp
                        for h in range(2):
                            m = gp * 2 + h
                            lhs = xz[32 * r:32 * r + 32,
                                     blk * 2 * B + h * B + q * BC:
                                     blk * 2 * B + h * B + q * BC + BC]
                            for half in range(2):
                                nc.tensor.matmul(
                                    ps[32 * m:32 * m + 32,
                                       half * 512:(half + 1) * 512],
                                    lhs,
                                    w[32 * r:32 * r + 32,
                                      half * 512:(half + 1) * 512],
                                    start=True, stop=True,
                                    tile_position=(32 * r, 32 * m),
                                )
                    dst = U[:, blk * 2 * JD + g2 * JD:
                            blk * 2 * JD + (g2 + 1) * JD]
                    if (blk * 2 + g2) % 4 < dve_drain_mod:
                        nc.vector.tensor_copy(dst, ps[:])
                    else:
                        nc.scalar.copy(dst, ps[:])
                if agree_lag is not None and (blk + 1) % SEC_BLKS == 0:
                    p1sec = (blk + 1) // SEC_BLKS - 1
                    asec = p1sec - agree_lag
                    if asec >= 0:
                        if not emitted_pre[0]:
                            emitted_pre[0] = True
                            if pre_agree is not None:
                                pre_agree()
                        agreement_section(1, asec)
            if agree_lag is not None:
                for asec in range(NSEC - agree_lag, NSEC):
                    agreement_section(1, asec)

        # ------------------------------------------------------ routing ops
        def agreement_section(t, sec):
                Us = U[:, sec * SEC_COLS:(sec + 1) * SEC_COLS].rearrange(
                    "p (bh d j) -> p bh d j", bh=SEC_BH, d=D_OUT, j=N_OUT)
                P = prodb[:].rearrange(
                    "p (bh d j) -> p bh d j", bh=SEC_BH, d=D_OUT, j=N_OUT)
                v4 = (v_bf[:]
                      .rearrange("p (d j) -> p d j", d=D_OUT, j=N_OUT)
                      .unsqueeze(1)
                      .to_broadcast((128, SEC_BH, D_OUT, N_OUT)))
                nc.vector.tensor_tensor(P, Us, v4, AluOp.mult)   # 2x
                # d-tree: 32 -> 16 -> 8 -> 4 -> 2 -> 1
                T1 = treeb[:, :4096].rearrange(
                    "p (bh d j) -> p bh d j", bh=SEC_BH, d=16, j=N_OUT)
                nc.vector.tensor_tensor(
                    T1, P[:, :, 0:16, :], P[:, :, 16:32, :], AluOp.add)
                T2 = prodb[:, :2048].rearrange(
                    "p (bh d j) -> p bh d j", bh=SEC_BH, d=8, j=N_OUT)
                nc.vector.tensor_tensor(
                    T2, T1[:, :, 0:8, :], T1[:, :, 8:16, :], AluOp.add)
                T3 = treeb[:, :1024].rearrange(
                    "p (bh d j) -> p bh d j", bh=SEC_BH, d=4, j=N_OUT)
                nc.vector.tensor_tensor(
                    T3, T2[:, :, 0:4, :], T2[:, :, 4:8, :], AluOp.add)
                T4 = prodb[:, 2048:2560].rearrange(
                    "p (bh d j) -> p bh d j", bh=SEC_BH, d=2, j=N_OUT)
                nc.vector.tensor_tensor(
                    T4, T3[:, :, 0:2, :], T3[:, :, 2:4, :], AluOp.add)
                a_cols = SEC_BH * N_OUT            # 256 logit cols / section
                if t == 1:
                    T5 = algA[:, sec * a_cols:(sec + 1) * a_cols].rearrange(
                        "p (bh d j) -> p bh d j", bh=SEC_BH, d=1, j=N_OUT)
                    nc.vector.tensor_tensor(
                        T5, T4[:, :, 0:1, :], T4[:, :, 1:2, :], AluOp.add)
                else:
                    T5 = tsec.rearrange(
                        "p (bh d j) -> p bh d j", bh=SEC_BH, d=1, j=N_OUT)
                    nc.vector.tensor_tensor(
                        T5, T4[:, :, 0:1, :], T4[:, :, 1:2, :], AluOp.add)
                    asl = algA[:, sec * a_cols:(sec + 1) * a_cols]
                    nc.vector.tensor_add(asl, asl, tsec)

        def softmax():
            nc.scalar.activation(ebuf, algA[:], Act.Exp)
            e3 = ebuf.rearrange("p (bh j) -> p bh j", bh=NBLK * 2, j=N_OUT)
            nc.vector.reduce_sum(Zt[:], e3, axis=Axis.X)
            nc.vector.reciprocal(Zr[:], Zt[:])
            c3 = cbuf[:].rearrange("p (bh j) -> p bh j", bh=NBLK * 2, j=N_OUT)
            zr3 = Zr[:].unsqueeze(2).to_broadcast((128, NBLK * 2, N_OUT))
            nc.vector.tensor_tensor(c3, e3, zr3, AluOp.mult)

        def weighted_sum(q, t):
            """s_partial = sum_i c * U.  The DVE only computes the c*U
            product (2x); ALL the summation (64 bh column-groups AND the 4
            partition groups) happens on the PE as one 128-matmul psum
            accumulation chain through the ones matrix."""
            ps_red = psrpool.tile([BC, JD], f32, tag="psred")
            NSW = 16                       # 2-blk product sections
            SW_BH = NBLK * 2 // NSW        # 4 (blk,g2) groups per section
            SW_COLS = SW_BH * JD           # 4096
            slots = [prodb[:, 0:SW_COLS], prodb[:, SW_COLS:2 * SW_COLS],
                     treeb[:, 0:SW_COLS]]
            for sec in range(NSW):
                slot = slots[sec % 3]
                P = slot.rearrange(
                    "p (bh d j) -> p bh d j", bh=SW_BH, d=D_OUT, j=N_OUT)
                Us = U[:, sec * SW_COLS:(sec + 1) * SW_COLS].rearrange(
                    "p (bh d j) -> p bh d j", bh=SW_BH, d=D_OUT, j=N_OUT)
                a_cols = SW_BH * N_OUT
                c4 = (cbuf[:, sec * a_cols:(sec + 1) * a_cols]
                      .rearrange("p (bh j) -> p bh j", bh=SW_BH, j=N_OUT)
                      .unsqueeze(2)
                      .to_broadcast((128, SW_BH, D_OUT, N_OUT)))
                nc.vector.tensor_tensor(P, Us, c4, AluOp.mult)   # 2x
                for bh in range(SW_BH):
                    for half in range(2):
                        nc.tensor.matmul(
                            ps_red[:, half * 512:(half + 1) * 512],
                            ones32[:],
                            slot[:, bh * JD + half * 512:
                                 bh * JD + half * 512 + 512],
                            start=(sec == 0 and bh == 0),
                            stop=(sec == NSW - 1 and bh == SW_BH - 1),
                        )
            nc.scalar.copy(s_sb[:], ps_red[:])
            ar_in, ar_out = ar_bufs[(q, t)]
            nc.gpsimd.dma_start(ar_in[:], s_sb[:])
            nc.gpsimd.collective_compute(
                "AllReduce", AluOp.add, replica_groups=rg,
                ins=[ar_in.opt()], outs=[ar_out.opt()],
            )
            return ar_out

        def squash(ar_src, row_off, final, q, dma_eng=None):
            """v = squash(s), DVE-only (no ACT/Sync queue head-blocking):
            rsqrt via fast-inverse-sqrt bit trick + 2 Newton steps.
            dma_eng picks the replicate-DMA queue: the gpsimd queue blocks on
            any collective emitted before it, so only the chunk-0 inline
            squash (whose sync queue is full of w-dmas at that point) uses
            gpsimd; everywhere else sync is free."""
            eng = nc.gpsimd if dma_eng == "g" else nc.sync
            for g in range(4):
                eng.dma_start(
                    s_rep[32 * g:32 * g + 32, :],
                    ar_src[row_off:row_off + BC, :],
                )
            nc.vector.tensor_mul(p2, s_rep[:], s_rep[:])
            p3 = p2.rearrange("p (d j) -> p j d", d=D_OUT, j=N_OUT)
            nc.vector.reduce_sum(s2[:], p3, axis=Axis.X)
            nc.vector.tensor_scalar_add(den[:], s2[:], 1.0 + EPS)
            nc.vector.tensor_scalar_add(lns[:], s2[:], EPS)   # x = s2+eps
            ii = lns[:].bitcast(mybir.dt.int32)
            # y0 = bits(0x5f3759df - (bits(x) >> 1))
            nc.vector.tensor_scalar(
                ibuf[:], ii, 1, None,
                mybir.AluOpType.logical_shift_right)
            nc.vector.tensor_scalar(
                ibuf[:], ibuf[:], 0x5F3759DF, -1,
                mybir.AluOpType.subtract, mybir.AluOpType.mult)
            y0 = ibuf[:].bitcast(f32)
            # two Newton steps: y <- y*(1.5 - 0.5*x*y^2)
            nc.vector.tensor_mul(rsq[:], y0, y0)
            nc.vector.tensor_mul(rsq[:], rsq[:], lns[:])
            nc.vector.tensor_scalar(
                rsq[:], rsq[:], -0.5, 1.5,
                mybir.AluOpType.mult, mybir.AluOpType.add)
            nc.vector.tensor_mul(rsq[:], rsq[:], y0)
            nc.vector.tensor_mul(fsc[:], rsq[:], rsq[:])
            nc.vector.tensor_mul(fsc[:], fsc[:], lns[:])
            nc.vector.tensor_scalar(
                fsc[:], fsc[:], -0.5, 1.5,
                mybir.AluOpType.mult, mybir.AluOpType.add)
            nc.vector.tensor_mul(rsq[:], rsq[:], fsc[:])
            nc.vector.reciprocal(rinv[:], den[:])
            nc.vector.tensor_mul(fsc[:], rsq[:], rinv[:])
            nc.vector.tensor_mul(fsc[:], fsc[:], s2[:])
            s3 = s_rep[:].rearrange("p (d j) -> p d j", d=D_OUT, j=N_OUT)
            if not final:
                f3 = fsc[:].unsqueeze(1).to_broadcast((128, D_OUT, N_OUT))
                v3 = v_bf[:].rearrange("p (d j) -> p d j", d=D_OUT, j=N_OUT)
                nc.vector.tensor_tensor(v3, s3, f3, AluOp.mult)
            else:
                # final output, reference layout v[b, j*32+d]
                vf = v_fin[:].rearrange("p (j d) -> p d j", j=N_OUT, d=D_OUT)
                nc.vector.tensor_tensor(
                    vf,
                    s_rep[0:BC, :].rearrange(
                        "p (d j) -> p d j", d=D_OUT, j=N_OUT),
                    fsc[0:BC, :].unsqueeze(1).to_broadcast(
                        (BC, D_OUT, N_OUT)),
                    AluOp.mult)
                nc.gpsimd.dma_start(v_out[q * BC:(q + 1) * BC, :], v_fin[:])

        def rounds(q):
            """Rounds t=1,2 for chunk q; the t=1 agreement was already
            emitted inline with phase 1, and the final squash is deferred to
            the caller (hides the last AllReduce's latency)."""
            ar_out = None
            for t in (1, 2):
                if t == 2:
                    for sec in range(NSEC):
                        agreement_section(t, sec)
                softmax()
                ar_out = weighted_sum(q, t)
                if t < 2:
                    squash(ar_out, 0, False, q)
            return ar_out

        # ------------------------------------------------------ emission
        s0_burst()
        nc.sync.dma_start(xz[:], xz_in[:])   # first needed by phase1 blk 0
        nc.scalar.copy(s0_sb[:], ps_s0[:])
        nc.gpsimd.dma_start(ar0_in[:], s0_sb[:])
        nc.gpsimd.collective_compute(
            "AllReduce", AluOp.add, replica_groups=rg,
            ins=[ar0_in.opt()], outs=[ar0_out.opt()],
        )
        # v0-squash for chunk 0 is emitted inside phase 1 right before the
        # first inlined agreement section, so the DVE drains the early
        # phase-1 psums while the s0 AllReduce is still in flight.
        phase1(0, agree_lag=3,
               pre_agree=lambda: squash(ar0_out, 0, False, 0, dma_eng="g"),
               dve_drain_mod=1)
        ar_last0 = rounds(0)
        squash(ar0_out, BC, False, 1)       # v0 for chunk 1
        # chunk 1 phase 1 overlaps chunk 0's final AllReduce latency
        phase1(1, agree_lag=0)
        squash(ar_last0, 0, True, 0)        # chunk 0 output
        ar_last1 = rounds(1)
        squash(ar_last1, 0, True, 1)

    _fix_sync_waits(nc)
    return nc


# ---------------------------------------------------------------- host prep
def _prep_inputs(x, W):
    """Per-core input maps.

    Local capsule l = blk*8 + g2*4 + gp*2 + h.
    SBUF rows r128 = g2*64 + gp*32 + hp*16 + k.
      wt[r128; blk*1024 + d*32 + j] = W[l(blk,g2,gp,hp), j, d, k]
      xz[r128; blk*128 + h*64 + b]  = x[b, l(blk,g2,gp,h), k] if hp==h else 0
      xd[r128; blk*64 + b]          = x[b, l(blk,g2,gp,hp), k] / 32
    """
    import jax.numpy as jnp

    def tobf(a):
        return np.asarray(jnp.asarray(a).astype(jnp.bfloat16))

    in_maps = []
    ones32 = np.zeros((128, BC), np.float32)
    for p in range(128):
        ones32[p, p % 32] = 1.0
    ones32 = tobf(ones32)
    for c in range(N_CORES):
        xi = x[:, c * I_LOC:(c + 1) * I_LOC, :]          # [B, 256, 16]
        wi = W[0, c * I_LOC:(c + 1) * I_LOC]             # [256, 32, 32, 16]
        # l = blk*8 + g2*4 + gp*2 + h
        x6 = xi.reshape(B, NBLK, 2, 2, 2, D_IN)          # b,blk,g2,gp,h,k
        w7 = wi.reshape(NBLK, 2, 2, 2, N_OUT, D_OUT, D_IN)  # blk,g2,gp,h,j,d,k

        # wt[(g2,gp,hp,k); (blk, d, j)]
        wt = np.transpose(w7, (1, 2, 3, 6, 0, 5, 4)).reshape(128, NBLK * JD)

        # xz[(g2,gp,hp,k); (blk, h, b)] with h-select zero interleave
        xt = np.transpose(x6, (2, 3, 4, 5, 1, 0))        # g2,gp,h,k,blk,b
        xz = np.zeros((2, 2, 2, D_IN, NBLK, 2, B), np.float32)
        for h in range(2):
            xz[:, :, h, :, :, h, :] = xt[:, :, h]
        xz = xz.reshape(128, NBLK * 2 * B)

        # xd[(g2,gp,hp,k); (blk, b)] = x/32 dense
        xd = (xt / 32.0).reshape(128, NBLK * B)

        in_maps.append({
            "wt": tobf(np.ascontiguousarray(wt)),
            "xz": tobf(np.ascontiguousarray(xz)),
            "xd": tobf(np.ascontiguousarray(xd)),
            "ones32": ones32,
        })
    return in_maps


_cached = {}


def _get_program():
    if "nc" not in _cached:
        _cached["nc"] = _build_program()
    return _cached["nc"]


def kernel(x, W):
    x = np.asarray(x, dtype=np.float32)
    W = np.asarray(W, dtype=np.float32)
    nc = _get_program()
    in_maps = _prep_inputs(x, W)
    res = bass_utils.run_bass_kernel_spmd(
        nc, in_maps, core_ids=list(range(N_CORES))
    )
    v = res.results[0]["v"].reshape(B, N_OUT, D_OUT)
    return v.astype(np.float32)
